# revision 11
# baseline (speedup 1.0000x reference)
# Trainium2 Bass kernel for nn_Encoder (6-layer conv-attention encoder).
# Sharding: 4 batch groups x 2-way sequence split. Each core owns one half of
# one batch element's sequence in "own coordinates" (cols = [t0-2, t0+514)).
# k/v are computed for the own half only and exchanged via one fused AllGather
# per layer; a tiny 4-column halo of the residual is exchanged per layer
# boundary. All SBUF data is fp16 (faster DVE modes, better precision than
# bf16); PSUM accumulation stays f32.
import sys
sys.path.insert(0, '/opt/trn_rl_repo')
import numpy as np

from concourse import bacc, tile, mybir
import concourse.bass as bass
from concourse.bass_utils import run_bass_kernel_spmd

B, C, T = 4, 512, 1024
F, KW, L, H = 2048, 3, 6, 8
KC, DR = 64, 32
TO, TE = 512, 516          # own cols / own+halo cols
NC8 = 8
F16 = mybir.dt.float16
F32 = mybir.dt.float32
AF = mybir.ActivationFunctionType
ALU = mybir.AluOpType
EPS = 1e-4
P = 128

NCH_E = ((0, 512), (512, 4))      # extent-516 psum-bank-aligned chunks
NCH_V = ((0, 512), (512, 8))      # vT 520

_CACHE = {}
TRACE = False
LAST_RESULT = None


def _emit(nc, tc, d, flags, n_layers=L, do_gather=True):
    (has_bv, ln1_aff, ln2_aff) = flags
    from contextlib import ExitStack
    ctx = ExitStack()

    def pool(name, bufs, space="SBUF"):
        return ctx.enter_context(tc.tile_pool(name=name, bufs=bufs, space=space))

    pers = pool("pers", 1)
    dram = pool("dram", 1, space="DRAM")
    p_psA = pool("psA", 2, space="PSUM")

    p_kraw = pool("kraw", 2)
    p_qraw = pool("qraw", 2)
    p_tmp = pool("ropetmp", 2)
    p_shuf = pool("shuf", 2)
    p_pt = pool("pt", 8)
    p_rbc = pool("rbc", 2)
    p_resid = pool("resid", 10)
    p_lntmp = pool("lntmp", 6)
    p_lndx = pool("lndx", 4)
    p_sq = pool("sq", 8)
    p_rstd = pool("rstd", 2)
    p_ht = pool("ht", 2)
    p_hm = pool("hm", 4)
    p_wq = pool("wq", 2)
    p_wk = pool("wk", 2)
    p_wv = pool("wv", 2)
    p_wo = pool("wo", 2)
    p_w1 = pool("w1", 3)
    p_w2 = pool("w2", 3)
    p_par = pool("par", 2)
    p_out = pool("outp", 1)
    p_halo = pool("halo", 2)

    x_t = [pers.tile([P, TE], F16, tag=f"x{m}", name=f"x{m}") for m in range(4)]
    kr_t = [pers.tile([P, T], F16, tag=f"kr{m}", name=f"kr{m}") for m in range(4)]
    q_t = [pers.tile([P, TE], F16, tag=f"q{m}", name=f"q{m}") for m in range(4)]
    ko_t = [pers.tile([P, TO], F16, tag=f"ko{m}", name=f"ko{m}") for m in range(4)]
    vt_t = [pers.tile([P, 520], F16, tag=f"vt{j}", name=f"vt{j}") for j in range(8)]
    vo_t = [pers.tile([P, 520], F16, tag=f"vo{j}", name=f"vo{j}") for j in range(4)]
    onorm_t = [pers.tile([P, TE], F16, tag=f"on{i}", name=f"on{i}") for i in range(4)]
    r2_t = [pers.tile([33, 520], F16, tag=f"r2{i}", name=f"r2{i}") for i in range(4)]
    x1b_t = [pers.tile([P, TE], F16, tag=f"x1b{m}", name=f"x1b{m}") for m in range(4)]
    SWAP_MASK = list(range(16, 32)) + list(range(0, 16))
    cos_k = pers.tile([P, TO], F16, tag="cosk", name="cosk")
    sin_k = pers.tile([P, TO], F16, tag="sink", name="sink")
    cos_q = pers.tile([P, TE], F16, tag="cosq", name="cosq")
    sin_q = pers.tile([P, TE], F16, tag="sinq", name="sinq")
    maskx = pers.tile([P, TE], F16, tag="maskx", name="maskx")
    maskh = pers.tile([P, 514], F16, tag="maskh", name="maskh")
    sel2 = pers.tile([33, 128], F16, tag="sel2", name="sel2")
    onesm = pers.tile([P, 256], F16, tag="onesm", name="onesm")
    eps_sb = pers.tile([P, 1], F32, tag="eps", name="eps")
    hcoef = pers.tile([P, 4], F32, tag="hcoef", name="hcoef")

    dma = nc.sync.dma_start
    for name, t in [("cos_k_d", cos_k), ("sin_k_d", sin_k), ("cos_q_d", cos_q),
                    ("sin_q_d", sin_q), ("maskx_d", maskx), ("maskh_d", maskh),
                    ("sel2_d", sel2), ("ones_d", onesm), ("hcoef_d", hcoef)]:
        dma(t[:, :], d[name][:, :])
    for m in range(4):
        dma(x_t[m][:, :], d["x0_d"][m * P:(m + 1) * P, :])
        nc.vector.memset(r2_t[m][:, :], 0.0)
    nc.vector.memset(eps_sb[:, :], EPS)

    mm = nc.tensor.matmul

    def mm_chunks(o, n):
        """Split a column range at the 512 psum-bank boundary."""
        out = []
        if o < 512:
            out.append((o, min(n, 512 - o)))
        if o + n > 512:
            oo = max(o, 512)
            out.append((oo, o + n - oo))
        return out

    def ln(xr_l, ext, chunks, par, affcols, out_l, rstd_mask=None, out_off=0):
        """Channel LayerNorm. xr_l: 4 [P, ext] f16 tiles. Writes out_l tiles
        at column offset out_off. chunks: column ranges processed as
        independent pipelined chains."""
        sum_ps = p_psA.tile([P, 1024], F32, tag="psA", name="psA")
        sq_ps = p_psA.tile([P, 1024], F32, tag="psA", name="psA")
        for (o, n) in chunks:
            for (oo, nn) in mm_chunks(o, n):
                for kk in range(4):
                    mm(sum_ps[:, oo:oo + nn], onesm[:, 0:128], xr_l[kk][:, oo:oo + nn],
                       start=(kk == 0), stop=(kk == 3), skip_group_check=True)
            sq_l = []
            for kk in range(4):
                sq = p_sq.tile([P, 520], F16, tag="sq", name="sq")
                nc.vector.tensor_mul(sq[:, o:o + n], xr_l[kk][:, o:o + n],
                                     xr_l[kk][:, o:o + n])
                sq_l.append(sq)
            for (oo, nn) in mm_chunks(o, n):
                for kk in range(4):
                    mm(sq_ps[:, oo:oo + nn], onesm[:, 128:256], sq_l[kk][:, oo:oo + nn],
                       start=(kk == 0), stop=(kk == 3), skip_group_check=True)
            mean2 = p_lntmp.tile([P, TE], F32, tag="lntmp", name="lntmp")
            nc.scalar.activation(mean2[:, o:o + n], sum_ps[:, o:o + n], AF.Square)
            var = p_lntmp.tile([P, TE], F32, tag="lntmp", name="lntmp")
            nc.vector.scalar_tensor_tensor(var[:, o:o + n], sq_ps[:, o:o + n], 1.0,
                                           mean2[:, o:o + n],
                                           op0=ALU.mult, op1=ALU.subtract)
            std = p_lntmp.tile([P, TE], F32, tag="lntmp", name="lntmp")
            nc.scalar.activation(std[:, o:o + n], var[:, o:o + n], AF.Sqrt,
                                 bias=eps_sb[:, 0:1])
            rstd = p_rstd.tile([P, TE], F16, tag="rstd", name="rstd")
            with nc.allow_low_precision(reason="LN 1/std in fp16 is within tolerance"):
                nc.vector.reciprocal(rstd[:, o:o + n], std[:, o:o + n])
            if rstd_mask is not None:
                nc.vector.tensor_mul(rstd[:, o:o + n], rstd[:, o:o + n],
                                     rstd_mask[:, o:o + n])
            for m in range(4):
                dx = p_lndx.tile([P, TE], F16, tag="lndx", name="lndx")
                nc.vector.tensor_add(dx[:, o:o + n], xr_l[m][:, o:o + n],
                                     sum_ps[:, o:o + n])
                oap = out_l[m][:, out_off + o:out_off + o + n]
                nc.vector.tensor_mul(oap, dx[:, o:o + n], rstd[:, o:o + n])
                if affcols is not None:
                    gc, bc_ = affcols
                    nc.scalar.activation(oap, out_l[m][:, out_off + o:out_off + o + n],
                                         AF.Identity, bias=par[:, bc_ + m:bc_ + m + 1],
                                         scale=par[:, gc + m:gc + m + 1])

    for li in range(n_layers):
        last = li == n_layers - 1
        wq = p_wq.tile([P, 2048], F16, tag="wq", name="wq")
        wk = p_wk.tile([P, 2048], F16, tag="wk", name="wk")
        wv = p_wv.tile([P, 4 * 520], F16, tag="wv", name="wv")
        wo = p_wo.tile([P, 2048], F16, tag="wo", name="wo")
        par = p_par.tile([P, 52], F32, tag="par", name="par")
        for t, dn in [(wk, "wk_d"), (wv, "wv_d"), (wq, "wq_d"), (wo, "wo_d"), (par, "par_d")]:
            dma(t[:, :], d[dn][li][:, :])

        bin_kv = dram.tile([C, 1032], F16, tag=f"bkv{li}", name=f"bkv{li}")
        bout_kv = dram.tile([2 * C, 1032], F16, tag=f"bokv{li}", name=f"bokv{li}")

        # ---- k own-half projection + rope ----
        for m in range(4):
            ps = p_psA.tile([P, 1024], F32, tag="psA", name="psA")
            for (o, n) in ((0, 258), (258, 254)):
                for kk in range(4):
                    mm(ps[:, o:o + n], wk[:, kk * 512 + m * P: kk * 512 + (m + 1) * P],
                       x_t[kk][:, 2 + o:2 + o + n], start=(kk == 0), stop=(kk == 3),
                       skip_group_check=True)
            kraw = p_kraw.tile([P, TO], F16, tag="kraw", name="kraw")
            nc.scalar.activation(kraw[:, :], ps[:, 0:TO], AF.Identity,
                                 bias=par[:, 4 + m:5 + m], scale=1.0)
            sh = p_shuf.tile([P, TO], F16, tag="shuf", name="shuf")
            nc.vector.stream_shuffle(sh[:, :], kraw[:, :], SWAP_MASK)
            t1 = p_tmp.tile([P, TO], F16, tag="ropetmp", name="ropetmp")
            t2 = p_tmp.tile([P, TO], F16, tag="ropetmp", name="ropetmp")
            nc.vector.tensor_mul(t1[:, :], kraw[:, :], cos_k[:, :])
            nc.vector.tensor_mul(t2[:, :], sh[:, :], sin_k[:, :])
            nc.vector.tensor_add(ko_t[m][:, :], t1[:, :], t2[:, :])
            dma(bin_kv[m * P:(m + 1) * P, 0:512], ko_t[m][:, :])

        # ---- v own-half (transposed, with ones column per head) ----
        for jj in range(4):
            ps = p_psA.tile([P, 1024], F32, tag="psA", name="psA")
            for (o, n) in NCH_V:
                for kk in range(4):
                    mm(ps[:, o:o + n], x_t[kk][:, 2 + jj * P:2 + (jj + 1) * P],
                       wv[:, kk * 520 + o: kk * 520 + o + n],
                       start=(kk == 0), stop=(kk == 3))
            nc.scalar.activation(vo_t[jj][:, :], ps[:, 0:520], AF.Copy)
            ones_ap = vo_t[jj][:, :].rearrange("p (h c) -> p h c", c=65)[:, :, 64:65]
            nc.vector.memset(ones_ap, 1.0)
            dma(bin_kv[jj * P:(jj + 1) * P, 512:1032], vo_t[jj][:, :])

        # ---- k/v exchange (own half <-> pair partner) ----
        if do_gather:
            nc.gpsimd.collective_compute(
                "AllGather", ALU.bypass,
                replica_groups=[[0, 1], [2, 3], [4, 5], [6, 7]],
                ins=[bin_kv[:, :].opt()], outs=[bout_kv[:, :].opt()])
            kv_src = bout_kv
        else:
            kv_src = bin_kv
        for m in range(4):
            dma(kr_t[m][:, 0:512], kv_src[m * P:(m + 1) * P, 0:512])
            if do_gather:
                dma(kr_t[m][:, 512:1024], kv_src[C + m * P:C + (m + 1) * P, 0:512])
            else:
                dma(kr_t[m][:, 512:1024], kv_src[m * P:(m + 1) * P, 0:512])
        for j in range(8):
            jj = j % 4
            src_r = jj * P if (j < 4 or not do_gather) else C + jj * P
            dma(vt_t[j][:, :], kv_src[src_r:src_r + P, 512:1032])

        # ---- q projection + rope + attention (q(i) fills pair boundaries) ----
        with tc.tile_pool(name="psO", bufs=2, space="PSUM") as p_psO:
            for i in range(4):  # head pairs
                m = i
                ps = p_psA.tile([P, 1024], F32, tag="psA", name="psA")
                for (o, n) in NCH_E:
                    for kk in range(4):
                        mm(ps[:, o:o + n], wq[:, kk * 512 + m * P: kk * 512 + (m + 1) * P],
                           x_t[kk][:, o:o + n], start=(kk == 0), stop=(kk == 3))
                qraw = p_qraw.tile([P, TE], F16, tag="qraw", name="qraw")
                nc.scalar.activation(qraw[:, :], ps[:, 0:TE], AF.Identity,
                                     bias=par[:, 0 + m:1 + m], scale=1.0)
                sh = p_shuf.tile([P, TE], F16, tag="shuf", name="shuf")
                nc.vector.stream_shuffle(sh[:, :], qraw[:, :], SWAP_MASK)
                t1 = p_tmp.tile([P, TE], F16, tag="ropetmp", name="ropetmp")
                t2 = p_tmp.tile([P, TE], F16, tag="ropetmp", name="ropetmp")
                nc.vector.tensor_mul(t1[:, :], qraw[:, :], cos_q[:, :])
                nc.vector.tensor_mul(t2[:, :], sh[:, :], sin_q[:, :])
                nc.vector.tensor_add(q_t[m][:, :], t1[:, :], t2[:, :])

                ops_pair = [p_psO.tile([65, TE], F32, tag="psO", name="psO")
                            for _ in range(2)]
                for j in range(8):
                    for sub in range(2):
                        hh = 2 * i + sub
                        o_ps = ops_pair[sub]
                        sc = p_psA.tile([P, 1024], F32, tag="psA", name="psA")
                        for (o, n) in NCH_E:
                            mm(sc[:, o:o + n],
                               kr_t[i][sub * 64:(sub + 1) * 64, j * P:(j + 1) * P],
                               q_t[i][sub * 64:(sub + 1) * 64, o:o + n],
                               start=True, stop=True)
                        pt = p_pt.tile([P, TE], F16, tag="pt", name="pt")
                        nc.scalar.activation(pt[:, :], sc[:, 0:TE], AF.Exp)
                        for (o, n) in NCH_E:
                            mm(o_ps[:, o:o + n], vt_t[j][:, hh * 65:(hh + 1) * 65],
                               pt[:, o:o + n], start=(j == 0), stop=(j == 7),
                               skip_group_check=True)
                for sub in range(2):
                    nc.vector.tensor_copy(r2_t[i][sub * 32:sub * 32 + 1, 0:TE],
                                          ops_pair[sub][64:65, 0:TE])
                bc_ps = p_psA.tile([P, 1024], F32, tag="psA", name="psA")
                for (o, n) in NCH_E:
                    mm(bc_ps[:, o:o + n], sel2[:, :], r2_t[i][:, o:o + n],
                       start=True, stop=True)
                rbc = p_rbc.tile([P, TE], F16, tag="rbc", name="rbc")
                with nc.allow_low_precision(reason="softmax 1/Z in fp16 is within tolerance"):
                    nc.vector.reciprocal(rbc[:, :], bc_ps[:, 0:TE])
                for sub in range(2):
                    nc.vector.tensor_mul(onorm_t[i][sub * 64:(sub + 1) * 64, :],
                                         ops_pair[sub][0:64, 0:TE],
                                         rbc[sub * 64:(sub + 1) * 64, :])
                    if has_bv:
                        nc.vector.tensor_scalar_add(
                            onorm_t[i][sub * 64:(sub + 1) * 64, :],
                            onorm_t[i][sub * 64:(sub + 1) * 64, :],
                            par[sub * 64:(sub + 1) * 64, 48 + i:49 + i])

        # ---- Wo + residual + LN1 ----
        xr_l = []
        for m in range(4):
            ps = p_psA.tile([P, 1024], F32, tag="psA", name="psA")
            for (o, n) in NCH_E:
                for kk in range(4):
                    mm(ps[:, o:o + n], wo[:, kk * 512 + m * P: kk * 512 + (m + 1) * P],
                       onorm_t[kk][:, o:o + n], start=(kk == 0), stop=(kk == 3))
            xr = p_resid.tile([P, TE], F16, tag="resid", name="resid")
            nc.vector.scalar_tensor_tensor(xr[:, :], ps[:, 0:TE], par[:, 8 + m:9 + m],
                                           x_t[m][:, :], op0=ALU.add, op1=ALU.add)
            xr_l.append(xr)
        ln(xr_l, TE, ((0, 260), (260, 256)), par, (32, 36) if ln1_aff else None,
           x1b_t, rstd_mask=maskx)

        # ---- FFN ----
        HCH = ((0, 258), (258, 256))  # h extent 514, pipelined halves
        with tc.tile_pool(name="psY", bufs=4, space="PSUM") as p_psY:
            y_ps = [p_psY.tile([P, 512], F32, tag="psY", name="psY") for m in range(4)]
            for fm in range(16):
                w1t = p_w1.tile([P, 12 * 128], F16, tag="w1", name="w1")
                dma(w1t[:, :], d["w1_d"][li][:, fm * 1536:(fm + 1) * 1536])
                h_ps = p_psA.tile([P, 1024], F32, tag="psA", name="psA")
                for (o, n) in HCH:
                    for (oo, nn) in mm_chunks(o, n):
                        bidx = 0
                        for kk in range(4):
                            for dk in range(3):
                                mm(h_ps[:, oo:oo + nn], w1t[:, bidx * 128:(bidx + 1) * 128],
                                   x1b_t[kk][:, dk + oo: dk + oo + nn],
                                   start=(bidx == 0), stop=(bidx == 11),
                                   skip_group_check=True)
                                bidx += 1
                ht = p_ht.tile([P, 514], F16, tag="ht", name="ht")
                nc.scalar.activation(ht[:, :], h_ps[:, 0:514], AF.Relu,
                                     bias=par[:, 12 + fm:13 + fm], scale=1.0)
                hm = p_hm.tile([P, 514], F16, tag="hm", name="hm")
                nc.vector.tensor_mul(hm[:, :], ht[:, :], maskh[:, :])
                w2t = p_w2.tile([P, 12 * 128], F16, tag="w2", name="w2")
                dma(w2t[:, :], d["w2_d"][li][:, fm * 1536:(fm + 1) * 1536])
                for m in range(4):
                    for dk in range(3):
                        mm(y_ps[m][:, 0:512], w2t[:, (m * 3 + dk) * 128:(m * 3 + dk + 1) * 128],
                           hm[:, dk:dk + 512],
                           start=(fm == 0 and dk == 0), stop=(fm == 15 and dk == 2),
                           skip_group_check=True)
            xr2_l = []
            for m in range(4):
                xr2 = p_resid.tile([P, TO], F16, tag="resid", name="resid")
                nc.vector.scalar_tensor_tensor(xr2[:, :], y_ps[m][:, 0:TO],
                                               par[:, 28 + m:29 + m],
                                               x1b_t[m][:, 2:2 + TO],
                                               op0=ALU.add, op1=ALU.add)
                xr2_l.append(xr2)
            if last:
                o32 = [p_out.tile([P, TO], F32, tag=f"o32{m}", name=f"o32{m}")
                       for m in range(4)]
                ln(xr2_l, TO, ((0, 258), (258, 254)), par,
                   (40, 44) if ln2_aff else None, o32)
                for m in range(4):
                    dma(d["out_d"][m * P:(m + 1) * P, :], o32[m][:, :])
            else:
                ln(xr2_l, TO, ((0, 258), (258, 254)), par,
                   (40, 44) if ln2_aff else None, x_t, out_off=2)

        # ---- residual halo exchange (4 boundary cols) ----
        if not last:
            bin_h = dram.tile([C, 4], F16, tag=f"bh{li}", name=f"bh{li}")
            bout_h = dram.tile([2 * C, 4], F16, tag=f"boh{li}", name=f"boh{li}")
            for m in range(4):
                dma(bin_h[m * P:(m + 1) * P, 0:2], x_t[m][:, 2:4])
                dma(bin_h[m * P:(m + 1) * P, 2:4], x_t[m][:, 512:514])
            if do_gather:
                nc.gpsimd.collective_compute(
                    "AllGather", ALU.bypass,
                    replica_groups=[[0, 1], [2, 3], [4, 5], [6, 7]],
                    ins=[bin_h[:, :].opt()], outs=[bout_h[:, :].opt()])
                h_src = bout_h
            else:
                h_src = bin_h
            for m in range(4):
                hA = p_halo.tile([P, 4], F16, tag="halo", name="halo")
                hB = p_halo.tile([P, 4], F16, tag="halo", name="halo")
                dma(hA[:, :], h_src[m * P:(m + 1) * P, :])
                if do_gather:
                    dma(hB[:, :], h_src[C + m * P:C + (m + 1) * P, :])
                else:
                    dma(hB[:, :], h_src[m * P:(m + 1) * P, :])
                # left halo cols 0:2  = hA.last2*cl0 + hB.last2*cl1
                tl = p_halo.tile([P, 4], F16, tag="halot", name="halot")
                nc.vector.tensor_scalar_mul(tl[:, 0:2], hA[:, 2:4], hcoef[:, 0:1])
                nc.vector.scalar_tensor_tensor(x_t[m][:, 0:2], hB[:, 2:4],
                                               hcoef[:, 1:2], tl[:, 0:2],
                                               op0=ALU.mult, op1=ALU.add)
                # right halo cols 514:516 = hA.first2*cr0 + hB.first2*cr1
                nc.vector.tensor_scalar_mul(tl[:, 2:4], hA[:, 0:2], hcoef[:, 2:3])
                nc.vector.scalar_tensor_tensor(x_t[m][:, 514:516], hB[:, 0:2],
                                               hcoef[:, 3:4], tl[:, 2:4],
                                               op0=ALU.mult, op1=ALU.add)

    ctx.close()


def build_program(flags, n_layers=L, do_gather=True):
    nc = bacc.Bacc(target_bir_lowering=False, trn_type="TRN2", num_devices=NC8)
    d = {}
    d["x0_d"] = nc.declare_dram_parameter("x0", [C, TE], F16, isOutput=False)
    d["cos_k_d"] = nc.declare_dram_parameter("cos_k", [128, TO], F16, isOutput=False)
    d["sin_k_d"] = nc.declare_dram_parameter("sin_k", [128, TO], F16, isOutput=False)
    d["cos_q_d"] = nc.declare_dram_parameter("cos_q", [128, TE], F16, isOutput=False)
    d["sin_q_d"] = nc.declare_dram_parameter("sin_q", [128, TE], F16, isOutput=False)
    d["maskx_d"] = nc.declare_dram_parameter("maskx", [128, TE], F16, isOutput=False)
    d["maskh_d"] = nc.declare_dram_parameter("maskh", [128, 514], F16, isOutput=False)
    d["hcoef_d"] = nc.declare_dram_parameter("hcoef", [128, 4], F32, isOutput=False)
    d["sel2_d"] = nc.declare_dram_parameter("sel2", [33, 128], F16, isOutput=False)
    d["ones_d"] = nc.declare_dram_parameter("onesmat", [128, 256], F16, isOutput=False)
    for key, shp, dt in [("wq_d", [128, 2048], F16), ("wk_d", [128, 2048], F16),
                         ("wv_d", [128, 4 * 520], F16), ("wo_d", [128, 2048], F16),
                         ("w1_d", [128, 16 * 12 * 128], F16),
                         ("w2_d", [128, 16 * 12 * 128], F16),
                         ("par_d", [128, 52], F32)]:
        d[key] = [nc.declare_dram_parameter(f"{key[:-2]}{i}", shp, dt, isOutput=False)
                  for i in range(L)]
    d["out_d"] = nc.declare_dram_parameter("out", [C, TO], F32, isOutput=True)
    with tile.TileContext(nc) as tc:
        _emit(nc, tc, d, flags, n_layers=n_layers, do_gather=do_gather)
    nc.compile()
    return nc


# ======================= host side =======================

def _rope_tables(tvals):
    theta = 1.0 / (10000.0 ** (np.arange(0, DR, 2) / DR))
    cos = np.ones((128, len(tvals)), np.float32)
    sin = np.zeros((128, len(tvals)), np.float32)
    for r in range(128):
        lc = r % 64
        if lc < 16:
            ang = theta[lc] * tvals
            cos[r] = np.cos(ang); sin[r] = -np.sin(ang)
        elif lc < 32:
            ang = theta[lc - 16] * tvals
            cos[r] = np.cos(ang); sin[r] = np.sin(ang)
    return cos, sin


def _f16(x):
    return np.ascontiguousarray(np.asarray(x, np.float32).astype(np.float16))


def _pack_weights(inputs):
    per_layer = []
    for li in range(L):
        Wq = np.asarray(inputs['Wq'][li][:, :, 0], np.float32) / 8.0
        Wk = np.asarray(inputs['Wk'][li][:, :, 0], np.float32)
        Wv = np.asarray(inputs['Wv'][li][:, :, 0], np.float32)
        Wo = np.asarray(inputs['Wo'][li][:, :, 0], np.float32)
        W1 = np.asarray(inputs['W1'][li], np.float32)  # [F, C, 3]
        W2 = np.asarray(inputs['W2'][li], np.float32)  # [C, F, 3]

        def packT(W):
            WT = W.T
            return np.concatenate([WT[kk * 128:(kk + 1) * 128, :] for kk in range(4)], axis=1)

        wq_p = packT(Wq); wk_p = packT(Wk); wo_p = packT(Wo)
        WvT = Wv.T
        wv_p = np.zeros((128, 4 * 520), np.float32)
        for kk in range(4):
            blk = WvT[kk * 128:(kk + 1) * 128, :]
            for hh in range(8):
                wv_p[:, kk * 520 + hh * 65: kk * 520 + hh * 65 + 64] = blk[:, hh * 64:(hh + 1) * 64]
        w1_p = np.zeros((128, 16 * 12 * 128), np.float32)
        for fm in range(16):
            for kk in range(4):
                for dk in range(3):
                    b = kk * 3 + dk
                    w1_p[:, fm * 1536 + b * 128: fm * 1536 + (b + 1) * 128] = \
                        W1[fm * 128:(fm + 1) * 128, kk * 128:(kk + 1) * 128, dk].T
        w2_p = np.zeros((128, 16 * 12 * 128), np.float32)
        for fk in range(16):
            for m in range(4):
                for dk in range(3):
                    b = fk * 12 + m * 3 + dk
                    w2_p[:, b * 128:(b + 1) * 128] = \
                        W2[m * 128:(m + 1) * 128, fk * 128:(fk + 1) * 128, dk].T
        par = np.zeros((128, 52), np.float32)

        def col4(vec):
            return np.asarray(vec, np.float32).reshape(4, 128).T

        par[:, 0:4] = col4(inputs['bq'][li]) / 8.0
        par[:, 4:8] = col4(inputs['bk'][li])
        par[:, 8:12] = col4(inputs['bo'][li])
        par[:, 12:28] = np.asarray(inputs['c1'][li], np.float32).reshape(16, 128).T
        par[:, 28:32] = col4(inputs['c2'][li])
        par[:, 32:36] = col4(inputs['g1'][li])
        par[:, 36:40] = col4(inputs['be1'][li])
        par[:, 40:44] = col4(inputs['g2'][li])
        par[:, 44:48] = col4(inputs['be2'][li])
        par[:, 48:52] = col4(inputs['bv'][li])
        per_layer.append(dict(wq=_f16(wq_p), wk=_f16(wk_p), wv=_f16(wv_p),
                              wo=_f16(wo_p), w1=_f16(w1_p), w2=_f16(w2_p), par=par))
    return per_layer


def kernel(**inputs):
    inputs = {k: np.asarray(v) for k, v in inputs.items()}
    x = inputs['x'].astype(np.float32) * inputs['x_mask'].astype(np.float32)
    has_bv = bool(np.any(inputs['bv'] != 0))
    ln1_aff = bool(np.any(inputs['g1'] != 1) or np.any(inputs['be1'] != 0))
    ln2_aff = bool(np.any(inputs['g2'] != 1) or np.any(inputs['be2'] != 0))
    flags = (has_bv, ln1_aff, ln2_aff)
    if flags not in _CACHE:
        _CACHE[flags] = build_program(flags)
    nc = _CACHE[flags]

    wl = _pack_weights(inputs)
    onesmat = np.concatenate([np.full((128, 128), -1.0 / 512, np.float32),
                              np.full((128, 128), 1.0 / 512, np.float32)], axis=1)
    sel2 = np.zeros((33, 128), np.float32)
    sel2[0, 0:64] = 1.0
    sel2[32, 64:128] = 1.0

    in_maps = []
    for core in range(NC8):
        g, h = core // 2, core % 2
        t0 = h * TO
        # own+halo window [t0-2, t0+514), zero-padded at sequence edges
        xp = np.zeros((C, TE), np.float32)
        lo, hi = max(t0 - 2, 0), min(t0 + 514, T)
        xp[:, lo - (t0 - 2):hi - (t0 - 2)] = x[g][:, lo:hi]
        cos_k, sin_k = _rope_tables(np.arange(t0, t0 + TO, dtype=np.float64))
        cos_q, sin_q = _rope_tables(np.arange(t0 - 2, t0 + 514, dtype=np.float64))
        mx = np.ones((128, TE), np.float32)
        mh = np.ones((128, 514), np.float32)
        if h == 0:
            mx[:, 0:2] = 0; mh[:, 0:1] = 0
            hc = np.array([0.0, 0.0, 0.0, 1.0], np.float32)   # cl0, cl1, cr0, cr1
        else:
            mx[:, 514:516] = 0; mh[:, 513:514] = 0
            hc = np.array([1.0, 0.0, 0.0, 0.0], np.float32)
        im = {
            "x0": _f16(xp),
            "cos_k": _f16(cos_k), "sin_k": _f16(sin_k),
            "cos_q": _f16(cos_q), "sin_q": _f16(sin_q),
            "maskx": _f16(mx), "maskh": _f16(mh),
            "hcoef": np.repeat(hc[None, :], 128, axis=0),
            "sel2": _f16(sel2), "onesmat": _f16(onesmat),
        }
        for li in range(L):
            w = wl[li]
            im[f"wq{li}"] = w['wq']; im[f"wk{li}"] = w['wk']
            im[f"wv{li}"] = w['wv']; im[f"wo{li}"] = w['wo']
            im[f"w1{li}"] = w['w1']; im[f"w2{li}"] = w['w2']
            im[f"par{li}"] = w['par']
        in_maps.append(im)

    global LAST_RESULT
    res = run_bass_kernel_spmd(nc, in_maps, core_ids=list(range(NC8)),
                               trace=TRACE)
    LAST_RESULT = res
    out = np.zeros((B, C, T), np.float32)
    for g in range(B):
        out[g, :, 0:TO] = res.results[2 * g]["out"]
        out[g, :, TO:T] = res.results[2 * g + 1]["out"]
    out_dt = np.asarray(inputs['x']).dtype
    return out.astype(out_dt)


# revision 15
# speedup vs baseline: 1.0364x; 1.0364x over previous
# Trainium2 Bass kernel for nn_Encoder (6-layer conv-attention encoder).
# Sharding: 4 batch groups x 2-way sequence split. Each core owns one half of
# one batch element's sequence in "own coordinates" (cols = [t0-2, t0+514)).
# k/v are computed for the own half only and exchanged via one fused AllGather
# per layer; a tiny 4-column halo of the residual is exchanged per layer
# boundary. All SBUF data is fp16 (faster DVE modes, better precision than
# bf16); PSUM accumulation stays f32.
import sys
sys.path.insert(0, '/opt/trn_rl_repo')
import numpy as np

from concourse import bacc, tile, mybir
import concourse.bass as bass
from concourse.bass_utils import run_bass_kernel_spmd

B, C, T = 4, 512, 1024
F, KW, L, H = 2048, 3, 6, 8
KC, DR = 64, 32
TO, TE = 512, 516          # own cols / own+halo cols
NC8 = 8
F16 = mybir.dt.float16
F32 = mybir.dt.float32
AF = mybir.ActivationFunctionType
ALU = mybir.AluOpType
EPS = 1e-4
P = 128

NCH_E = ((0, 512), (512, 4))      # extent-516 psum-bank-aligned chunks
NCH_V = ((0, 512), (512, 8))      # vT 520

_CACHE = {}
TRACE = False
LAST_RESULT = None


def _emit(nc, tc, d, flags, n_layers=L, do_gather=True):
    (has_bv, ln1_aff, ln2_aff) = flags
    from contextlib import ExitStack
    ctx = ExitStack()

    def pool(name, bufs, space="SBUF"):
        return ctx.enter_context(tc.tile_pool(name=name, bufs=bufs, space=space))

    pers = pool("pers", 1)
    dram = pool("dram", 1, space="DRAM")
    p_psA = pool("psA", 2, space="PSUM")

    p_kraw = pool("kraw", 2)
    p_qraw = pool("qraw", 2)
    p_tmp = pool("ropetmp", 2)
    p_shuf = pool("shuf", 2)
    p_pt = pool("pt", 8)
    p_rbc = pool("rbc", 2)
    p_resid = pool("resid", 10)
    p_lntmp = pool("lntmp", 6)
    p_lndx = pool("lndx", 4)
    p_sq = pool("sq", 8)
    p_rstd = pool("rstd", 2)
    p_ht = pool("ht", 2)
    p_hm = pool("hm", 4)
    p_wq = pool("wq", 2)
    p_wk = pool("wk", 2)
    p_wv = pool("wv", 2)
    p_wo = pool("wo", 2)
    p_w1 = pool("w1", 3)
    p_w2 = pool("w2", 3)
    p_par = pool("par", 2)
    p_out = pool("outp", 1)
    p_halo = pool("halo", 2)

    x_t = [pers.tile([P, TE], F16, tag=f"x{m}", name=f"x{m}") for m in range(4)]
    kr_t = [pers.tile([P, T], F16, tag=f"kr{m}", name=f"kr{m}") for m in range(4)]
    q_t = [pers.tile([P, TE], F16, tag=f"q{m}", name=f"q{m}") for m in range(4)]
    vt_t = [pers.tile([P, 520], F16, tag=f"vt{j}", name=f"vt{j}") for j in range(8)]
    onorm_t = [pers.tile([P, TE], F16, tag=f"on{i}", name=f"on{i}") for i in range(4)]
    r2_t = [pers.tile([33, 520], F16, tag=f"r2{i}", name=f"r2{i}") for i in range(4)]
    x1b_t = [pers.tile([P, TE], F16, tag=f"x1b{m}", name=f"x1b{m}") for m in range(4)]
    SWAP_MASK = list(range(16, 32)) + list(range(0, 16))
    cos_k = pers.tile([P, TO], F16, tag="cosk", name="cosk")
    sin_k = pers.tile([P, TO], F16, tag="sink", name="sink")
    cos_q = pers.tile([P, TE], F16, tag="cosq", name="cosq")
    sin_q = pers.tile([P, TE], F16, tag="sinq", name="sinq")
    maskx = pers.tile([P, TE], F16, tag="maskx", name="maskx")
    maskh = pers.tile([P, 514], F16, tag="maskh", name="maskh")
    sel2 = pers.tile([33, 128], F16, tag="sel2", name="sel2")
    onesm = pers.tile([P, 256], F16, tag="onesm", name="onesm")
    eps_sb = pers.tile([P, 1], F32, tag="eps", name="eps")
    hcoef = pers.tile([P, 4], F32, tag="hcoef", name="hcoef")

    dma = nc.sync.dma_start
    for name, t in [("cos_k_d", cos_k), ("sin_k_d", sin_k), ("cos_q_d", cos_q),
                    ("sin_q_d", sin_q), ("maskx_d", maskx), ("maskh_d", maskh),
                    ("sel2_d", sel2), ("ones_d", onesm), ("hcoef_d", hcoef)]:
        dma(t[:, :], d[name][:, :])
    for m in range(4):
        dma(x_t[m][:, :], d["x0_d"][m * P:(m + 1) * P, :])
        nc.vector.memset(r2_t[m][:, :], 0.0)
    nc.vector.memset(eps_sb[:, :], EPS)

    mm = nc.tensor.matmul

    def mm_chunks(o, n):
        """Split a column range at the 512 psum-bank boundary."""
        out = []
        if o < 512:
            out.append((o, min(n, 512 - o)))
        if o + n > 512:
            oo = max(o, 512)
            out.append((oo, o + n - oo))
        return out

    def ln(xr_l, ext, chunks, par, affcols, out_l, rstd_mask=None, out_off=0):
        """Channel LayerNorm. xr_l: 4 [P, ext] f16 tiles. Writes out_l tiles
        at column offset out_off. chunks: column ranges processed as
        independent pipelined chains."""
        sum_ps = p_psA.tile([P, 1024], F32, tag="psA", name="psA")
        sq_ps = p_psA.tile([P, 1024], F32, tag="psA", name="psA")
        for (o, n) in chunks:
            for (oo, nn) in mm_chunks(o, n):
                for kk in range(4):
                    mm(sum_ps[:, oo:oo + nn], onesm[:, 0:128], xr_l[kk][:, oo:oo + nn],
                       start=(kk == 0), stop=(kk == 3), skip_group_check=True)
            sq_l = []
            for kk in range(4):
                sq = p_sq.tile([P, 520], F16, tag="sq", name="sq")
                nc.vector.tensor_mul(sq[:, o:o + n], xr_l[kk][:, o:o + n],
                                     xr_l[kk][:, o:o + n])
                sq_l.append(sq)
            for (oo, nn) in mm_chunks(o, n):
                for kk in range(4):
                    mm(sq_ps[:, oo:oo + nn], onesm[:, 128:256], sq_l[kk][:, oo:oo + nn],
                       start=(kk == 0), stop=(kk == 3), skip_group_check=True)
            mean2 = p_lntmp.tile([P, TE], F32, tag="lntmp", name="lntmp")
            nc.scalar.activation(mean2[:, o:o + n], sum_ps[:, o:o + n], AF.Square)
            var = p_lntmp.tile([P, TE], F32, tag="lntmp", name="lntmp")
            nc.vector.scalar_tensor_tensor(var[:, o:o + n], sq_ps[:, o:o + n], 1.0,
                                           mean2[:, o:o + n],
                                           op0=ALU.mult, op1=ALU.subtract)
            std = p_lntmp.tile([P, TE], F32, tag="lntmp", name="lntmp")
            nc.scalar.activation(std[:, o:o + n], var[:, o:o + n], AF.Sqrt,
                                 bias=eps_sb[:, 0:1])
            rstd = p_rstd.tile([P, TE], F16, tag="rstd", name="rstd")
            with nc.allow_low_precision(reason="LN 1/std in fp16 is within tolerance"):
                nc.vector.reciprocal(rstd[:, o:o + n], std[:, o:o + n])
            if rstd_mask is not None:
                nc.vector.tensor_mul(rstd[:, o:o + n], rstd[:, o:o + n],
                                     rstd_mask[:, o:o + n])
            for m in range(4):
                dx = p_lndx.tile([P, TE], F16, tag="lndx", name="lndx")
                nc.vector.tensor_add(dx[:, o:o + n], xr_l[m][:, o:o + n],
                                     sum_ps[:, o:o + n])
                oap = out_l[m][:, out_off + o:out_off + o + n]
                nc.vector.tensor_mul(oap, dx[:, o:o + n], rstd[:, o:o + n])
                if affcols is not None:
                    gc, bc_ = affcols
                    nc.scalar.activation(oap, out_l[m][:, out_off + o:out_off + o + n],
                                         AF.Identity, bias=par[:, bc_ + m:bc_ + m + 1],
                                         scale=par[:, gc + m:gc + m + 1])

    for li in range(n_layers):
        last = li == n_layers - 1
        wq = p_wq.tile([P, 2048], F16, tag="wq", name="wq")
        wk = p_wk.tile([P, 2048], F16, tag="wk", name="wk")
        wv = p_wv.tile([P, 4 * 520], F16, tag="wv", name="wv")
        wo = p_wo.tile([P, 2048], F16, tag="wo", name="wo")
        par = p_par.tile([P, 52], F32, tag="par", name="par")
        for t, dn in [(wk, "wk_d"), (wv, "wv_d"), (wq, "wq_d"), (wo, "wo_d"), (par, "par_d")]:
            dma(t[:, :], d[dn][li][:, :])

        bin_kv = dram.tile([C, 1032], F16, tag=f"bkv{li}", name=f"bkv{li}")
        bout_kv = dram.tile([C, 1032], F16, tag=f"bokv{li}", name=f"bokv{li}")

        # Own keys occupy chunks 0..3 of kr_t/vt_t directly (chunk order is
        # arbitrary as long as kr columns match vt rows); the partner half
        # arrives via AllReduce(sum) and a local subtract: partner = sum - own.
        # ---- k own-half projection + rope (written into kr_t[:, 0:512]) ----
        for m in range(4):
            ps = p_psA.tile([P, 1024], F32, tag="psA", name="psA")
            for (o, n) in ((0, 258), (258, 254)):
                for kk in range(4):
                    mm(ps[:, o:o + n], wk[:, kk * 512 + m * P: kk * 512 + (m + 1) * P],
                       x_t[kk][:, 2 + o:2 + o + n], start=(kk == 0), stop=(kk == 3),
                       skip_group_check=True)
            kraw = p_kraw.tile([P, TO], F16, tag="kraw", name="kraw")
            nc.scalar.activation(kraw[:, :], ps[:, 0:TO], AF.Identity,
                                 bias=par[:, 4 + m:5 + m], scale=1.0)
            sh = p_shuf.tile([P, TO], F16, tag="shuf", name="shuf")
            nc.vector.stream_shuffle(sh[:, :], kraw[:, :], SWAP_MASK)
            t1 = p_tmp.tile([P, TO], F16, tag="ropetmp", name="ropetmp")
            t2 = p_tmp.tile([P, TO], F16, tag="ropetmp", name="ropetmp")
            nc.vector.tensor_mul(t1[:, :], kraw[:, :], cos_k[:, :])
            nc.vector.tensor_mul(t2[:, :], sh[:, :], sin_k[:, :])
            nc.vector.tensor_add(kr_t[m][:, 0:TO], t1[:, :], t2[:, :])
            dma(bin_kv[m * P:(m + 1) * P, 0:512], kr_t[m][:, 0:TO])

        # ---- v own-half (transposed, with ones column per head) ----
        for jj in range(4):
            ps = p_psA.tile([P, 1024], F32, tag="psA", name="psA")
            for (o, n) in NCH_V:
                for kk in range(4):
                    mm(ps[:, o:o + n], x_t[kk][:, 2 + jj * P:2 + (jj + 1) * P],
                       wv[:, kk * 520 + o: kk * 520 + o + n],
                       start=(kk == 0), stop=(kk == 3))
            nc.scalar.activation(vt_t[jj][:, :], ps[:, 0:520], AF.Copy)
            ones_ap = vt_t[jj][:, :].rearrange("p (h c) -> p h c", c=65)[:, :, 64:65]
            nc.vector.memset(ones_ap, 1.0)
            dma(bin_kv[jj * P:(jj + 1) * P, 512:1032], vt_t[jj][:, :])

        # ---- k/v exchange: AllReduce(sum) over the pair, partner = sum - own
        if do_gather:
            nc.gpsimd.collective_compute(
                "AllReduce", ALU.add,
                replica_groups=[[0, 1], [2, 3], [4, 5], [6, 7]],
                ins=[bin_kv[:, :].opt()], outs=[bout_kv[:, :].opt()])
            kv_src = bout_kv
        else:
            kv_src = bin_kv
        for m in range(4):
            ksum = p_kraw.tile([P, TO], F16, tag="ksum", name="ksum")
            dma(ksum[:, :], kv_src[m * P:(m + 1) * P, 0:512])
            nc.vector.tensor_sub(kr_t[m][:, 512:1024], ksum[:, :], kr_t[m][:, 0:TO])
        for jj in range(4):
            vsum = p_kraw.tile([P, 520], F16, tag="vsum", name="vsum")
            dma(vsum[:, :], kv_src[jj * P:(jj + 1) * P, 512:1032])
            nc.vector.tensor_sub(vt_t[4 + jj][:, :], vsum[:, :], vt_t[jj][:, :])

        # ---- q projection + rope + attention (q(i) fills pair boundaries) ----
        with tc.tile_pool(name="psO", bufs=2, space="PSUM") as p_psO:
            for i in range(4):  # head pairs
                m = i
                ps = p_psA.tile([P, 1024], F32, tag="psA", name="psA")
                for (o, n) in NCH_E:
                    for kk in range(4):
                        mm(ps[:, o:o + n], wq[:, kk * 512 + m * P: kk * 512 + (m + 1) * P],
                           x_t[kk][:, o:o + n], start=(kk == 0), stop=(kk == 3))
                qraw = p_qraw.tile([P, TE], F16, tag="qraw", name="qraw")
                nc.scalar.activation(qraw[:, :], ps[:, 0:TE], AF.Identity,
                                     bias=par[:, 0 + m:1 + m], scale=1.0)
                sh = p_shuf.tile([P, TE], F16, tag="shuf", name="shuf")
                nc.vector.stream_shuffle(sh[:, :], qraw[:, :], SWAP_MASK)
                t1 = p_tmp.tile([P, TE], F16, tag="ropetmp", name="ropetmp")
                t2 = p_tmp.tile([P, TE], F16, tag="ropetmp", name="ropetmp")
                nc.vector.tensor_mul(t1[:, :], qraw[:, :], cos_q[:, :])
                nc.vector.tensor_mul(t2[:, :], sh[:, :], sin_q[:, :])
                nc.vector.tensor_add(q_t[m][:, :], t1[:, :], t2[:, :])

                ops_pair = [p_psO.tile([65, TE], F32, tag="psO", name="psO")
                            for _ in range(2)]
                for j in range(8):
                    for sub in range(2):
                        hh = 2 * i + sub
                        o_ps = ops_pair[sub]
                        sc = p_psA.tile([P, 1024], F32, tag="psA", name="psA")
                        for (o, n) in NCH_E:
                            mm(sc[:, o:o + n],
                               kr_t[i][sub * 64:(sub + 1) * 64, j * P:(j + 1) * P],
                               q_t[i][sub * 64:(sub + 1) * 64, o:o + n],
                               start=True, stop=True)
                        pt = p_pt.tile([P, TE], F16, tag="pt", name="pt")
                        nc.scalar.activation(pt[:, :], sc[:, 0:TE], AF.Exp)
                        for (o, n) in NCH_E:
                            mm(o_ps[:, o:o + n], vt_t[j][:, hh * 65:(hh + 1) * 65],
                               pt[:, o:o + n], start=(j == 0), stop=(j == 7),
                               skip_group_check=True)
                for sub in range(2):
                    nc.vector.tensor_copy(r2_t[i][sub * 32:sub * 32 + 1, 0:TE],
                                          ops_pair[sub][64:65, 0:TE])
                bc_ps = p_psA.tile([P, 1024], F32, tag="psA", name="psA")
                for (o, n) in NCH_E:
                    mm(bc_ps[:, o:o + n], sel2[:, :], r2_t[i][:, o:o + n],
                       start=True, stop=True)
                rbc = p_rbc.tile([P, TE], F16, tag="rbc", name="rbc")
                with nc.allow_low_precision(reason="softmax 1/Z in fp16 is within tolerance"):
                    nc.vector.reciprocal(rbc[:, :], bc_ps[:, 0:TE])
                for sub in range(2):
                    nc.vector.tensor_mul(onorm_t[i][sub * 64:(sub + 1) * 64, :],
                                         ops_pair[sub][0:64, 0:TE],
                                         rbc[sub * 64:(sub + 1) * 64, :])
                    if has_bv:
                        nc.vector.tensor_scalar_add(
                            onorm_t[i][sub * 64:(sub + 1) * 64, :],
                            onorm_t[i][sub * 64:(sub + 1) * 64, :],
                            par[sub * 64:(sub + 1) * 64, 48 + i:49 + i])

        # ---- Wo + residual + LN1 ----
        xr_l = []
        for m in range(4):
            ps = p_psA.tile([P, 1024], F32, tag="psA", name="psA")
            for (o, n) in NCH_E:
                for kk in range(4):
                    mm(ps[:, o:o + n], wo[:, kk * 512 + m * P: kk * 512 + (m + 1) * P],
                       onorm_t[kk][:, o:o + n], start=(kk == 0), stop=(kk == 3))
            xr = p_resid.tile([P, TE], F16, tag="resid", name="resid")
            nc.vector.scalar_tensor_tensor(xr[:, :], ps[:, 0:TE], par[:, 8 + m:9 + m],
                                           x_t[m][:, :], op0=ALU.add, op1=ALU.add)
            xr_l.append(xr)
        ln(xr_l, TE, ((0, 260), (260, 256)), par, (32, 36) if ln1_aff else None,
           x1b_t, rstd_mask=maskx)

        # ---- FFN ----
        HCH = ((0, 258), (258, 256))  # h extent 514, pipelined halves
        with tc.tile_pool(name="psY", bufs=4, space="PSUM") as p_psY:
            y_ps = [p_psY.tile([P, 512], F32, tag="psY", name="psY") for m in range(4)]
            for fm in range(16):
                w1t = p_w1.tile([P, 12 * 128], F16, tag="w1", name="w1")
                dma(w1t[:, :], d["w1_d"][li][:, fm * 1536:(fm + 1) * 1536])
                h_ps = p_psA.tile([P, 1024], F32, tag="psA", name="psA")
                for (o, n) in HCH:
                    for (oo, nn) in mm_chunks(o, n):
                        bidx = 0
                        for kk in range(4):
                            for dk in range(3):
                                mm(h_ps[:, oo:oo + nn], w1t[:, bidx * 128:(bidx + 1) * 128],
                                   x1b_t[kk][:, dk + oo: dk + oo + nn],
                                   start=(bidx == 0), stop=(bidx == 11),
                                   skip_group_check=True)
                                bidx += 1
                ht = p_ht.tile([P, 514], F16, tag="ht", name="ht")
                nc.scalar.activation(ht[:, :], h_ps[:, 0:514], AF.Relu,
                                     bias=par[:, 12 + fm:13 + fm], scale=1.0)
                hm = p_hm.tile([P, 514], F16, tag="hm", name="hm")
                nc.vector.tensor_mul(hm[:, :], ht[:, :], maskh[:, :])
                w2t = p_w2.tile([P, 12 * 128], F16, tag="w2", name="w2")
                dma(w2t[:, :], d["w2_d"][li][:, fm * 1536:(fm + 1) * 1536])
                for m in range(4):
                    for dk in range(3):
                        mm(y_ps[m][:, 0:512], w2t[:, (m * 3 + dk) * 128:(m * 3 + dk + 1) * 128],
                           hm[:, dk:dk + 512],
                           start=(fm == 0 and dk == 0), stop=(fm == 15 and dk == 2),
                           skip_group_check=True)
            xr2_l = []
            for m in range(4):
                xr2 = p_resid.tile([P, TO], F16, tag="resid", name="resid")
                nc.vector.scalar_tensor_tensor(xr2[:, :], y_ps[m][:, 0:TO],
                                               par[:, 28 + m:29 + m],
                                               x1b_t[m][:, 2:2 + TO],
                                               op0=ALU.add, op1=ALU.add)
                xr2_l.append(xr2)
            if last:
                o32 = [p_out.tile([P, TO], F32, tag=f"o32{m}", name=f"o32{m}")
                       for m in range(4)]
                ln(xr2_l, TO, ((0, 258), (258, 254)), par,
                   (40, 44) if ln2_aff else None, o32)
                for m in range(4):
                    dma(d["out_d"][m * P:(m + 1) * P, :], o32[m][:, :])
            else:
                ln(xr2_l, TO, ((0, 258), (258, 254)), par,
                   (40, 44) if ln2_aff else None, x_t, out_off=2)

        # ---- residual halo exchange (4 boundary cols, AllReduce+subtract) ----
        if not last:
            bin_h = dram.tile([C, 4], F16, tag=f"bh{li}", name=f"bh{li}")
            bout_h = dram.tile([C, 4], F16, tag=f"boh{li}", name=f"boh{li}")
            for m in range(4):
                dma(bin_h[m * P:(m + 1) * P, 0:2], x_t[m][:, 2:4])
                dma(bin_h[m * P:(m + 1) * P, 2:4], x_t[m][:, 512:514])
            if do_gather:
                nc.gpsimd.collective_compute(
                    "AllReduce", ALU.add,
                    replica_groups=[[0, 1], [2, 3], [4, 5], [6, 7]],
                    ins=[bin_h[:, :].opt()], outs=[bout_h[:, :].opt()])
                h_src = bout_h
            else:
                h_src = bin_h
            for m in range(4):
                hS = p_halo.tile([P, 4], F16, tag="halo", name="halo")
                dma(hS[:, :], h_src[m * P:(m + 1) * P, :])
                # partner.(first2,last2) = hS - own; select valid side via hcoef
                pd = p_halo.tile([P, 4], F16, tag="halot", name="halot")
                nc.vector.tensor_sub(pd[:, 0:2], hS[:, 2:4], x_t[m][:, 512:514])
                nc.vector.tensor_sub(pd[:, 2:4], hS[:, 0:2], x_t[m][:, 2:4])
                # left halo cols 0:2 = partner.last2*cl ; right = partner.first2*cr
                nc.vector.tensor_scalar_mul(x_t[m][:, 0:2], pd[:, 0:2], hcoef[:, 0:1])
                nc.vector.tensor_scalar_mul(x_t[m][:, 514:516], pd[:, 2:4], hcoef[:, 2:3])

    ctx.close()


def build_program(flags, n_layers=L, do_gather=True):
    nc = bacc.Bacc(target_bir_lowering=False, trn_type="TRN2", num_devices=NC8)
    d = {}
    d["x0_d"] = nc.declare_dram_parameter("x0", [C, TE], F16, isOutput=False)
    d["cos_k_d"] = nc.declare_dram_parameter("cos_k", [128, TO], F16, isOutput=False)
    d["sin_k_d"] = nc.declare_dram_parameter("sin_k", [128, TO], F16, isOutput=False)
    d["cos_q_d"] = nc.declare_dram_parameter("cos_q", [128, TE], F16, isOutput=False)
    d["sin_q_d"] = nc.declare_dram_parameter("sin_q", [128, TE], F16, isOutput=False)
    d["maskx_d"] = nc.declare_dram_parameter("maskx", [128, TE], F16, isOutput=False)
    d["maskh_d"] = nc.declare_dram_parameter("maskh", [128, 514], F16, isOutput=False)
    d["hcoef_d"] = nc.declare_dram_parameter("hcoef", [128, 4], F32, isOutput=False)
    d["sel2_d"] = nc.declare_dram_parameter("sel2", [33, 128], F16, isOutput=False)
    d["ones_d"] = nc.declare_dram_parameter("onesmat", [128, 256], F16, isOutput=False)
    for key, shp, dt in [("wq_d", [128, 2048], F16), ("wk_d", [128, 2048], F16),
                         ("wv_d", [128, 4 * 520], F16), ("wo_d", [128, 2048], F16),
                         ("w1_d", [128, 16 * 12 * 128], F16),
                         ("w2_d", [128, 16 * 12 * 128], F16),
                         ("par_d", [128, 52], F32)]:
        d[key] = [nc.declare_dram_parameter(f"{key[:-2]}{i}", shp, dt, isOutput=False)
                  for i in range(L)]
    d["out_d"] = nc.declare_dram_parameter("out", [C, TO], F32, isOutput=True)
    with tile.TileContext(nc) as tc:
        _emit(nc, tc, d, flags, n_layers=n_layers, do_gather=do_gather)
    nc.compile()
    return nc


# ======================= host side =======================

def _rope_tables(tvals):
    theta = 1.0 / (10000.0 ** (np.arange(0, DR, 2) / DR))
    cos = np.ones((128, len(tvals)), np.float32)
    sin = np.zeros((128, len(tvals)), np.float32)
    for r in range(128):
        lc = r % 64
        if lc < 16:
            ang = theta[lc] * tvals
            cos[r] = np.cos(ang); sin[r] = -np.sin(ang)
        elif lc < 32:
            ang = theta[lc - 16] * tvals
            cos[r] = np.cos(ang); sin[r] = np.sin(ang)
    return cos, sin


def _f16(x):
    return np.ascontiguousarray(np.asarray(x, np.float32).astype(np.float16))


def _pack_weights(inputs):
    per_layer = []
    for li in range(L):
        Wq = np.asarray(inputs['Wq'][li][:, :, 0], np.float32) / 8.0
        Wk = np.asarray(inputs['Wk'][li][:, :, 0], np.float32)
        Wv = np.asarray(inputs['Wv'][li][:, :, 0], np.float32)
        Wo = np.asarray(inputs['Wo'][li][:, :, 0], np.float32)
        W1 = np.asarray(inputs['W1'][li], np.float32)  # [F, C, 3]
        W2 = np.asarray(inputs['W2'][li], np.float32)  # [C, F, 3]

        def packT(W):
            WT = W.T
            return np.concatenate([WT[kk * 128:(kk + 1) * 128, :] for kk in range(4)], axis=1)

        wq_p = packT(Wq); wk_p = packT(Wk); wo_p = packT(Wo)
        WvT = Wv.T
        wv_p = np.zeros((128, 4 * 520), np.float32)
        for kk in range(4):
            blk = WvT[kk * 128:(kk + 1) * 128, :]
            for hh in range(8):
                wv_p[:, kk * 520 + hh * 65: kk * 520 + hh * 65 + 64] = blk[:, hh * 64:(hh + 1) * 64]
        w1_p = np.zeros((128, 16 * 12 * 128), np.float32)
        for fm in range(16):
            for kk in range(4):
                for dk in range(3):
                    b = kk * 3 + dk
                    w1_p[:, fm * 1536 + b * 128: fm * 1536 + (b + 1) * 128] = \
                        W1[fm * 128:(fm + 1) * 128, kk * 128:(kk + 1) * 128, dk].T
        w2_p = np.zeros((128, 16 * 12 * 128), np.float32)
        for fk in range(16):
            for m in range(4):
                for dk in range(3):
                    b = fk * 12 + m * 3 + dk
                    w2_p[:, b * 128:(b + 1) * 128] = \
                        W2[m * 128:(m + 1) * 128, fk * 128:(fk + 1) * 128, dk].T
        par = np.zeros((128, 52), np.float32)

        def col4(vec):
            return np.asarray(vec, np.float32).reshape(4, 128).T

        par[:, 0:4] = col4(inputs['bq'][li]) / 8.0
        par[:, 4:8] = col4(inputs['bk'][li])
        par[:, 8:12] = col4(inputs['bo'][li])
        par[:, 12:28] = np.asarray(inputs['c1'][li], np.float32).reshape(16, 128).T
        par[:, 28:32] = col4(inputs['c2'][li])
        par[:, 32:36] = col4(inputs['g1'][li])
        par[:, 36:40] = col4(inputs['be1'][li])
        par[:, 40:44] = col4(inputs['g2'][li])
        par[:, 44:48] = col4(inputs['be2'][li])
        par[:, 48:52] = col4(inputs['bv'][li])
        per_layer.append(dict(wq=_f16(wq_p), wk=_f16(wk_p), wv=_f16(wv_p),
                              wo=_f16(wo_p), w1=_f16(w1_p), w2=_f16(w2_p), par=par))
    return per_layer


def kernel(**inputs):
    inputs = {k: np.asarray(v) for k, v in inputs.items()}
    x = inputs['x'].astype(np.float32) * inputs['x_mask'].astype(np.float32)
    has_bv = bool(np.any(inputs['bv'] != 0))
    ln1_aff = bool(np.any(inputs['g1'] != 1) or np.any(inputs['be1'] != 0))
    ln2_aff = bool(np.any(inputs['g2'] != 1) or np.any(inputs['be2'] != 0))
    flags = (has_bv, ln1_aff, ln2_aff)
    if flags not in _CACHE:
        _CACHE[flags] = build_program(flags)
    nc = _CACHE[flags]

    wl = _pack_weights(inputs)
    onesmat = np.concatenate([np.full((128, 128), -1.0 / 512, np.float32),
                              np.full((128, 128), 1.0 / 512, np.float32)], axis=1)
    sel2 = np.zeros((33, 128), np.float32)
    sel2[0, 0:64] = 1.0
    sel2[32, 64:128] = 1.0

    in_maps = []
    for core in range(NC8):
        g, h = core // 2, core % 2
        t0 = h * TO
        # own+halo window [t0-2, t0+514), zero-padded at sequence edges
        xp = np.zeros((C, TE), np.float32)
        lo, hi = max(t0 - 2, 0), min(t0 + 514, T)
        xp[:, lo - (t0 - 2):hi - (t0 - 2)] = x[g][:, lo:hi]
        cos_k, sin_k = _rope_tables(np.arange(t0, t0 + TO, dtype=np.float64))
        cos_q, sin_q = _rope_tables(np.arange(t0 - 2, t0 + 514, dtype=np.float64))
        mx = np.ones((128, TE), np.float32)
        mh = np.ones((128, 514), np.float32)
        if h == 0:
            mx[:, 0:2] = 0; mh[:, 0:1] = 0
            hc = np.array([0.0, 0.0, 1.0, 0.0], np.float32)   # cl, -, cr, -
        else:
            mx[:, 514:516] = 0; mh[:, 513:514] = 0
            hc = np.array([1.0, 0.0, 0.0, 0.0], np.float32)
        im = {
            "x0": _f16(xp),
            "cos_k": _f16(cos_k), "sin_k": _f16(sin_k),
            "cos_q": _f16(cos_q), "sin_q": _f16(sin_q),
            "maskx": _f16(mx), "maskh": _f16(mh),
            "hcoef": np.repeat(hc[None, :], 128, axis=0),
            "sel2": _f16(sel2), "onesmat": _f16(onesmat),
        }
        for li in range(L):
            w = wl[li]
            im[f"wq{li}"] = w['wq']; im[f"wk{li}"] = w['wk']
            im[f"wv{li}"] = w['wv']; im[f"wo{li}"] = w['wo']
            im[f"w1{li}"] = w['w1']; im[f"w2{li}"] = w['w2']
            im[f"par{li}"] = w['par']
        in_maps.append(im)

    global LAST_RESULT
    res = run_bass_kernel_spmd(nc, in_maps, core_ids=list(range(NC8)),
                               trace=TRACE)
    LAST_RESULT = res
    out = np.zeros((B, C, T), np.float32)
    for g in range(B):
        out[g, :, 0:TO] = res.results[2 * g]["out"]
        out[g, :, TO:T] = res.results[2 * g + 1]["out"]
    out_dt = np.asarray(inputs['x']).dtype
    return out.astype(out_dt)


# revision 16
# speedup vs baseline: 1.1466x; 1.1064x over previous
# Trainium2 Bass kernel for nn_Encoder (6-layer conv-attention encoder).
# Sharding: 4 batch groups x 2-way sequence split. Each core owns one half of
# one batch element's sequence (512 columns, no halo in the residual). k/v are
# computed for the own half only; the partner half arrives via one fused
# AllReduce(sum) + local subtract per layer (partner = sum - own). The FFN's
# conv halo (4 columns of post-LN1 x1b) is exchanged the same way mid-layer.
# All SBUF data is fp16 (2x/4x DVE modes, better precision than bf16); PSUM
# accumulation stays f32.
import sys
sys.path.insert(0, '/opt/trn_rl_repo')
import numpy as np

from concourse import bacc, tile, mybir
import concourse.bass as bass
from concourse.bass_utils import run_bass_kernel_spmd

B, C, T = 4, 512, 1024
F, KW, L, H = 2048, 3, 6, 8
KC, DR = 64, 32
TO = 512                   # own cols
NC8 = 8
F16 = mybir.dt.float16
F32 = mybir.dt.float32
AF = mybir.ActivationFunctionType
ALU = mybir.AluOpType
EPS = 1e-4
P = 128

_CACHE = {}
TRACE = False
LAST_RESULT = None


def _emit(nc, tc, d, flags, n_layers=L, do_gather=True):
    (has_bv, ln1_aff, ln2_aff) = flags
    from contextlib import ExitStack
    ctx = ExitStack()

    def pool(name, bufs, space="SBUF"):
        return ctx.enter_context(tc.tile_pool(name=name, bufs=bufs, space=space))

    pers = pool("pers", 1)
    dram = pool("dram", 1, space="DRAM")

    p_kraw = pool("kraw", 2)
    p_qraw = pool("qraw", 2)
    p_tmp = pool("ropetmp", 3)
    p_shuf = pool("shuf", 2)
    p_pt = pool("pt", 8)
    p_rbc = pool("rbc", 4)
    p_resid = pool("resid", 10)
    p_lntmp = pool("lntmp", 6)
    p_lndx = pool("lndx", 4)
    p_sq = pool("sq", 8)
    p_rstd = pool("rstd", 2)
    p_ht = pool("ht", 2)
    p_hm = pool("hm", 4)
    p_wq = pool("wq", 2)
    p_wk = pool("wk", 2)
    p_wv = pool("wv", 2)
    p_wo = pool("wo", 2)
    p_w1 = pool("w1", 3)
    p_w2 = pool("w2", 3)
    p_par = pool("par", 2)
    p_out = pool("outp", 1)
    p_halo = pool("halo", 2)
    p_ksum = pool("ksum", 4)

    x_t = [pers.tile([P, TO], F16, tag=f"x{m}", name=f"x{m}") for m in range(4)]
    kr_t = [pers.tile([P, T], F16, tag=f"kr{m}", name=f"kr{m}") for m in range(4)]
    q_t = [pers.tile([P, TO], F16, tag=f"q{m}", name=f"q{m}") for m in range(4)]
    vt_t = [pers.tile([P, 520], F16, tag=f"vt{j}", name=f"vt{j}") for j in range(8)]
    onorm_t = [pers.tile([P, TO], F16, tag=f"on{i}", name=f"on{i}") for i in range(4)]
    rz_t = [pers.tile([65, 520], F16, tag=f"rz{s}", name=f"rz{s}") for s in range(2)]
    ones64 = pers.tile([65, 64], F16, tag="ones64", name="ones64")
    x1b_t = [pers.tile([P, 516], F16, tag=f"x1b{m}", name=f"x1b{m}") for m in range(4)]
    SWAP_MASK = list(range(16, 32)) + list(range(0, 16))
    cos_k = pers.tile([P, TO], F16, tag="cosk", name="cosk")
    sin_k = pers.tile([P, TO], F16, tag="sink", name="sink")
    maskh = pers.tile([P, 514], F16, tag="maskh", name="maskh")
    onesm = pers.tile([P, 256], F16, tag="onesm", name="onesm")
    eps_sb = pers.tile([P, 1], F32, tag="eps", name="eps")
    hcoef = pers.tile([P, 4], F32, tag="hcoef", name="hcoef")

    dma = nc.sync.dma_start
    for name, t in [("cos_k_d", cos_k), ("sin_k_d", sin_k), ("maskh_d", maskh),
                    ("ones_d", onesm), ("hcoef_d", hcoef)]:
        dma(t[:, :], d[name][:, :])
    for m in range(4):
        dma(x_t[m][:, :], d["x0_d"][m * P:(m + 1) * P, :])
    nc.vector.memset(eps_sb[:, :], EPS)
    for s in range(2):
        nc.vector.memset(rz_t[s][0:64, :], 0.0)
    nc.vector.memset(ones64[0:64, :], 0.0)
    nc.vector.memset(ones64[64:65, :], 1.0)

    mm = nc.tensor.matmul

    def ln(xr_l, chunks, par, affcols, out_l, out_off=0):
        """Channel LayerNorm over 512 columns. xr_l: 4 [P, 512] f16 tiles.
        Writes out_l tiles at column offset out_off. chunks: column ranges
        processed as independent pipelined chains."""
        sum_ps = p_psA.tile([P, 1024], F32, tag="psA", name="psA")
        sq_ps = p_psA.tile([P, 1024], F32, tag="psA", name="psA")
        for (o, n) in chunks:
            for kk in range(4):
                mm(sum_ps[:, o:o + n], onesm[:, 0:128], xr_l[kk][:, o:o + n],
                   start=(kk == 0), stop=(kk == 3), skip_group_check=True)
            sq_l = []
            for kk in range(4):
                sq = p_sq.tile([P, TO], F16, tag="sq", name="sq")
                nc.vector.tensor_mul(sq[:, o:o + n], xr_l[kk][:, o:o + n],
                                     xr_l[kk][:, o:o + n])
                sq_l.append(sq)
            for kk in range(4):
                mm(sq_ps[:, o:o + n], onesm[:, 128:256], sq_l[kk][:, o:o + n],
                   start=(kk == 0), stop=(kk == 3), skip_group_check=True)
            mean2 = p_lntmp.tile([P, TO], F32, tag="lntmp", name="lntmp")
            nc.scalar.activation(mean2[:, o:o + n], sum_ps[:, o:o + n], AF.Square)
            var = p_lntmp.tile([P, TO], F32, tag="lntmp", name="lntmp")
            nc.vector.scalar_tensor_tensor(var[:, o:o + n], sq_ps[:, o:o + n], 1.0,
                                           mean2[:, o:o + n],
                                           op0=ALU.mult, op1=ALU.subtract)
            std = p_lntmp.tile([P, TO], F32, tag="lntmp", name="lntmp")
            nc.scalar.activation(std[:, o:o + n], var[:, o:o + n], AF.Sqrt,
                                 bias=eps_sb[:, 0:1])
            rstd = p_rstd.tile([P, TO], F16, tag="rstd", name="rstd")
            with nc.allow_low_precision(reason="LN 1/std in fp16 is within tolerance"):
                nc.vector.reciprocal(rstd[:, o:o + n], std[:, o:o + n])
            for m in range(4):
                dx = p_lndx.tile([P, TO], F16, tag="lndx", name="lndx")
                nc.vector.tensor_add(dx[:, o:o + n], xr_l[m][:, o:o + n],
                                     sum_ps[:, o:o + n])
                oap = out_l[m][:, out_off + o:out_off + o + n]
                nc.vector.tensor_mul(oap, dx[:, o:o + n], rstd[:, o:o + n])
                if affcols is not None:
                    gc, bc_ = affcols
                    nc.scalar.activation(oap, out_l[m][:, out_off + o:out_off + o + n],
                                         AF.Identity, bias=par[:, bc_ + m:bc_ + m + 1],
                                         scale=par[:, gc + m:gc + m + 1])

    KCH = ((0, 258), (258, 254))
    for li in range(n_layers):
        last = li == n_layers - 1
        wq = p_wq.tile([P, 2048], F16, tag="wq", name="wq")
        wk = p_wk.tile([P, 2048], F16, tag="wk", name="wk")
        wv = p_wv.tile([P, 2048], F16, tag="wv", name="wv")
        wo = p_wo.tile([P, 2048], F16, tag="wo", name="wo")
        par = p_par.tile([P, 52], F32, tag="par", name="par")
        for t, dn in [(wk, "wk_d"), (wv, "wv_d"), (wq, "wq_d"), (wo, "wo_d"), (par, "par_d")]:
            dma(t[:, :], d[dn][li][:, :])

        bin_kv = dram.tile([C, 1024], F16, tag=f"bkv{li}", name=f"bkv{li}")
        bout_kv = dram.tile([C, 1024], F16, tag=f"bokv{li}", name=f"bokv{li}")

        # Own keys occupy chunks 0..3 of kr_t/vt_t directly (chunk order is
        # arbitrary as long as kr columns match vt rows); the partner half
        # arrives via AllReduce(sum) and subtract on Pool: partner = sum - own.
        with tc.tile_pool(name="psP", bufs=3, space="PSUM") as p_psP:
            # ---- k own-half projection + rope (into kr_t[:, 0:512]) ----
            for m in range(4):
                ps = p_psP.tile([P, 1024], F32, tag="psP", name="psP")
                for (o, n) in KCH:
                    for kk in range(4):
                        mm(ps[:, o:o + n], wk[:, kk * 512 + m * P: kk * 512 + (m + 1) * P],
                           x_t[kk][:, o:o + n], start=(kk == 0), stop=(kk == 3),
                           skip_group_check=True)
                kraw = p_kraw.tile([P, TO], F16, tag="kraw", name="kraw")
                nc.scalar.activation(kraw[:, :], ps[:, 0:TO], AF.Identity,
                                     bias=par[:, 4 + m:5 + m], scale=1.0)
                sh = p_shuf.tile([P, TO], F16, tag="shuf", name="shuf")
                nc.vector.stream_shuffle(sh[:, :], kraw[:, :], SWAP_MASK)
                t1 = p_tmp.tile([P, TO], F16, tag="ropetmp", name="ropetmp")
                t2 = p_tmp.tile([P, TO], F16, tag="ropetmp", name="ropetmp")
                nc.gpsimd.tensor_mul(t1[:, :], kraw[:, :], cos_k[:, :])
                nc.vector.tensor_mul(t2[:, :], sh[:, :], sin_k[:, :])
                nc.vector.tensor_add(kr_t[m][:, 0:TO], t1[:, :], t2[:, :])
                dma(bin_kv[m * P:(m + 1) * P, 0:512], kr_t[m][:, 0:TO])

            # ---- v own-half (transposed; ones blocks pre-set at col 64/head)
            for jj in range(4):
                ps = p_psP.tile([P, 1024], F32, tag="psP", name="psP")
                for kk in range(4):
                    mm(ps[:, 0:512], x_t[kk][:, jj * P:(jj + 1) * P],
                       wv[:, kk * 512: (kk + 1) * 512],
                       start=(kk == 0), stop=(kk == 3))
                vt3 = vt_t[jj][:, :].rearrange("p (h c) -> p h c", c=65)
                ps3 = ps[:, 0:512].rearrange("p (h c) -> p h c", c=64)
                nc.scalar.activation(vt3[:, :, 0:64], ps3[:, :, :], AF.Copy)
                if li == 0:
                    nc.vector.memset(vt3[:, :, 64:65], 1.0)
                    vt3b = vt_t[4 + jj][:, :].rearrange("p (h c) -> p h c", c=65)
                    nc.vector.memset(vt3b[:, :, 64:65], 1.0)
                dma(bin_kv[jj * P:(jj + 1) * P, 512:1024],
                    vt3[:, :, 0:64])

            # ---- k/v exchange: AllReduce(sum) over the pair ----
            if do_gather:
                nc.gpsimd.collective_compute(
                    "AllReduce", ALU.add,
                    replica_groups=[[0, 1], [2, 3], [4, 5], [6, 7]],
                    ins=[bin_kv[:, :].opt()], outs=[bout_kv[:, :].opt()])
                kv_src = bout_kv
            else:
                kv_src = bin_kv
            for m in range(4):
                ksum = p_ksum.tile([P, TO], F16, tag="ksum", name="ksum")
                dma(ksum[:, :], kv_src[m * P:(m + 1) * P, 0:512])
                nc.gpsimd.tensor_sub(kr_t[m][:, 512:1024], ksum[:, :], kr_t[m][:, 0:TO])
            for jj in range(4):
                vsum = p_ksum.tile([P, TO], F16, tag="vsum", name="vsum")
                dma(vsum[:, :], kv_src[jj * P:(jj + 1) * P, 512:1024])
                vs3 = vsum[:, :].rearrange("p (h c) -> p h c", c=64)
                va = vt_t[jj][:, :].rearrange("p (h c) -> p h c", c=65)
                vb = vt_t[4 + jj][:, :].rearrange("p (h c) -> p h c", c=65)
                nc.gpsimd.tensor_sub(vb[:, :, 0:64], vs3[:, :, :], va[:, :, 0:64])

            # ---- q projection + rope (overlaps the exchange) ----
            for m in range(4):
                ps = p_psP.tile([P, 1024], F32, tag="psP", name="psP")
                for kk in range(4):
                    mm(ps[:, 0:512], wq[:, kk * 512 + m * P: kk * 512 + (m + 1) * P],
                       x_t[kk][:, :], start=(kk == 0), stop=(kk == 3))
                qraw = p_qraw.tile([P, TO], F16, tag="qraw", name="qraw")
                nc.scalar.activation(qraw[:, :], ps[:, 0:TO], AF.Identity,
                                     bias=par[:, 0 + m:1 + m], scale=1.0)
                sh = p_shuf.tile([P, TO], F16, tag="shuf", name="shuf")
                nc.vector.stream_shuffle(sh[:, :], qraw[:, :], SWAP_MASK)
                t1 = p_tmp.tile([P, TO], F16, tag="ropetmp", name="ropetmp")
                t2 = p_tmp.tile([P, TO], F16, tag="ropetmp", name="ropetmp")
                nc.gpsimd.tensor_mul(t1[:, :], qraw[:, :], cos_k[:, :])
                nc.vector.tensor_mul(t2[:, :], sh[:, :], sin_k[:, :])
                nc.vector.tensor_add(q_t[m][:, :], t1[:, :], t2[:, :])

        # ---- attention: sc tiles are 1 PSUM bank; deep pipelining ----
        with tc.tile_pool(name="psS", bufs=5, space="PSUM") as p_psS, \
             tc.tile_pool(name="psO", bufs=3, space="PSUM") as p_psO:
            for i in range(4):  # head pairs
                ops_pair = [p_psO.tile([65, TO], F32, tag="psO", name="psO")
                            for _ in range(2)]
                for j in range(8):
                    for sub in range(2):
                        hh = 2 * i + sub
                        o_ps = ops_pair[sub]
                        sc = p_psS.tile([P, TO], F32, tag="psS", name="psS")
                        mm(sc[:, :],
                           kr_t[i][sub * 64:(sub + 1) * 64, j * P:(j + 1) * P],
                           q_t[i][sub * 64:(sub + 1) * 64, :],
                           start=True, stop=True)
                        pt = p_pt.tile([P, TO], F16, tag="pt", name="pt")
                        nc.scalar.activation(pt[:, :], sc[:, :], AF.Exp)
                        mm(o_ps[:, :], vt_t[j][:, hh * 65:(hh + 1) * 65],
                           pt[:, :], start=(j == 0), stop=(j == 7),
                           skip_group_check=True)
                for sub in range(2):
                    o_ps = ops_pair[sub]
                    # rz row 64 = 1/Z; rows 0:64 are zero (set once at init)
                    with nc.allow_low_precision(reason="softmax 1/Z in fp16"):
                        nc.vector.reciprocal(rz_t[sub][64:65, 0:TO], o_ps[64:65, 0:TO])
                    bc_ps = p_psS.tile([P, TO], F32, tag="psS", name="psS")
                    mm(bc_ps[0:64, :], ones64[:, :], rz_t[sub][:, 0:TO],
                       start=True, stop=True)
                    nc.vector.tensor_mul(onorm_t[i][sub * 64:(sub + 1) * 64, :],
                                         o_ps[0:64, :], bc_ps[0:64, :])
                    if has_bv:
                        nc.vector.tensor_scalar_add(
                            onorm_t[i][sub * 64:(sub + 1) * 64, :],
                            onorm_t[i][sub * 64:(sub + 1) * 64, :],
                            par[sub * 64:(sub + 1) * 64, 48 + i:49 + i])

        # ---- Wo + residual + LN1 + FFN ----
        p_psA = None
        with tc.tile_pool(name="psA", bufs=2, space="PSUM") as p_psA:
            xr_l = []
            for m in range(4):
                ps = p_psA.tile([P, 1024], F32, tag="psA", name="psA")
                for kk in range(4):
                    mm(ps[:, 0:512], wo[:, kk * 512 + m * P: kk * 512 + (m + 1) * P],
                       onorm_t[kk][:, :], start=(kk == 0), stop=(kk == 3))
                xr = p_resid.tile([P, TO], F16, tag="resid", name="resid")
                nc.vector.scalar_tensor_tensor(xr[:, :], ps[:, 0:TO], par[:, 8 + m:9 + m],
                                               x_t[m][:, :], op0=ALU.add, op1=ALU.add)
                xr_l.append(xr)
            ln(xr_l, KCH, par, (32, 36) if ln1_aff else None, x1b_t, out_off=2)

            # ---- x1b conv-halo exchange (4 cols, AllReduce+subtract) ----
            bin_h = dram.tile([C, 4], F16, tag=f"bh{li}", name=f"bh{li}")
            bout_h = dram.tile([C, 4], F16, tag=f"boh{li}", name=f"boh{li}")
            for m in range(4):
                dma(bin_h[m * P:(m + 1) * P, 0:2], x1b_t[m][:, 2:4])
                dma(bin_h[m * P:(m + 1) * P, 2:4], x1b_t[m][:, 512:514])
            if do_gather:
                nc.gpsimd.collective_compute(
                    "AllReduce", ALU.add,
                    replica_groups=[[0, 1], [2, 3], [4, 5], [6, 7]],
                    ins=[bin_h[:, :].opt()], outs=[bout_h[:, :].opt()])
                h_src = bout_h
            else:
                h_src = bin_h
            for m in range(4):
                hS = p_halo.tile([P, 4], F16, tag="halo", name="halo")
                dma(hS[:, :], h_src[m * P:(m + 1) * P, :])
                pd = p_halo.tile([P, 4], F16, tag="halot", name="halot")
                nc.vector.tensor_sub(pd[:, 0:2], hS[:, 2:4], x1b_t[m][:, 512:514])
                nc.vector.tensor_sub(pd[:, 2:4], hS[:, 0:2], x1b_t[m][:, 2:4])
                nc.vector.tensor_scalar_mul(x1b_t[m][:, 0:2], pd[:, 0:2], hcoef[:, 0:1])
                nc.vector.tensor_scalar_mul(x1b_t[m][:, 514:516], pd[:, 2:4], hcoef[:, 2:3])

            # ---- FFN ----
            # h tile index i = own position i-1 (conv halo via x1b cols 0:2/514:516).
            # Main chunk (2,508) uses own x1b only and starts right after LN1.
            HCH = ((2, 508), (0, 2), (510, 2), (512, 2))
            with tc.tile_pool(name="psY", bufs=4, space="PSUM") as p_psY:
                y_ps = [p_psY.tile([P, TO], F32, tag="psY", name="psY") for m in range(4)]
                for fm in range(16):
                    w1t = p_w1.tile([P, 12 * 128], F16, tag="w1", name="w1")
                    dma(w1t[:, :], d["w1_d"][li][:, fm * 1536:(fm + 1) * 1536])
                    h_ps = p_psA.tile([P, 1024], F32, tag="psA", name="psA")
                    for (o, n) in HCH:
                        bidx = 0
                        for kk in range(4):
                            for dk in range(3):
                                mm(h_ps[:, o:o + n], w1t[:, bidx * 128:(bidx + 1) * 128],
                                   x1b_t[kk][:, dk + o: dk + o + n],
                                   start=(bidx == 0), stop=(bidx == 11),
                                   skip_group_check=True)
                                bidx += 1
                    ht = p_ht.tile([P, 514], F16, tag="ht", name="ht")
                    nc.scalar.activation(ht[:, :], h_ps[:, 0:514], AF.Relu,
                                         bias=par[:, 12 + fm:13 + fm], scale=1.0)
                    hm = p_hm.tile([P, 514], F16, tag="hm", name="hm")
                    nc.vector.tensor_mul(hm[:, :], ht[:, :], maskh[:, :])
                    w2t = p_w2.tile([P, 12 * 128], F16, tag="w2", name="w2")
                    dma(w2t[:, :], d["w2_d"][li][:, fm * 1536:(fm + 1) * 1536])
                    for m in range(4):
                        for dk in range(3):
                            mm(y_ps[m][:, :], w2t[:, (m * 3 + dk) * 128:(m * 3 + dk + 1) * 128],
                               hm[:, dk:dk + 512],
                               start=(fm == 0 and dk == 0), stop=(fm == 15 and dk == 2),
                               skip_group_check=True)
                xr2_l = []
                for m in range(4):
                    xr2 = p_resid.tile([P, TO], F16, tag="resid", name="resid")
                    nc.vector.scalar_tensor_tensor(xr2[:, :], y_ps[m][:, :],
                                                   par[:, 28 + m:29 + m],
                                                   x1b_t[m][:, 2:2 + TO],
                                                   op0=ALU.add, op1=ALU.add)
                    xr2_l.append(xr2)
                if last:
                    o32 = [p_out.tile([P, TO], F32, tag=f"o32{m}", name=f"o32{m}")
                           for m in range(4)]
                    ln(xr2_l, KCH, par, (40, 44) if ln2_aff else None, o32)
                    for m in range(4):
                        dma(d["out_d"][m * P:(m + 1) * P, :], o32[m][:, :])
                else:
                    ln(xr2_l, KCH, par, (40, 44) if ln2_aff else None, x_t)

    ctx.close()


def build_program(flags, n_layers=L, do_gather=True):
    nc = bacc.Bacc(target_bir_lowering=False, trn_type="TRN2", num_devices=NC8)
    d = {}
    d["x0_d"] = nc.declare_dram_parameter("x0", [C, TO], F16, isOutput=False)
    d["cos_k_d"] = nc.declare_dram_parameter("cos_k", [128, TO], F16, isOutput=False)
    d["sin_k_d"] = nc.declare_dram_parameter("sin_k", [128, TO], F16, isOutput=False)
    d["maskh_d"] = nc.declare_dram_parameter("maskh", [128, 514], F16, isOutput=False)
    d["hcoef_d"] = nc.declare_dram_parameter("hcoef", [128, 4], F32, isOutput=False)
    d["ones_d"] = nc.declare_dram_parameter("onesmat", [128, 256], F16, isOutput=False)
    for key, shp, dt in [("wq_d", [128, 2048], F16), ("wk_d", [128, 2048], F16),
                         ("wv_d", [128, 2048], F16), ("wo_d", [128, 2048], F16),
                         ("w1_d", [128, 16 * 12 * 128], F16),
                         ("w2_d", [128, 16 * 12 * 128], F16),
                         ("par_d", [128, 52], F32)]:
        d[key] = [nc.declare_dram_parameter(f"{key[:-2]}{i}", shp, dt, isOutput=False)
                  for i in range(L)]
    d["out_d"] = nc.declare_dram_parameter("out", [C, TO], F32, isOutput=True)
    with tile.TileContext(nc) as tc:
        _emit(nc, tc, d, flags, n_layers=n_layers, do_gather=do_gather)
    nc.compile()
    return nc


# ======================= host side =======================

def _rope_tables(tvals):
    theta = 1.0 / (10000.0 ** (np.arange(0, DR, 2) / DR))
    cos = np.ones((128, len(tvals)), np.float32)
    sin = np.zeros((128, len(tvals)), np.float32)
    for r in range(128):
        lc = r % 64
        if lc < 16:
            ang = theta[lc] * tvals
            cos[r] = np.cos(ang); sin[r] = -np.sin(ang)
        elif lc < 32:
            ang = theta[lc - 16] * tvals
            cos[r] = np.cos(ang); sin[r] = np.sin(ang)
    return cos, sin


def _f16(x):
    return np.ascontiguousarray(np.asarray(x, np.float32).astype(np.float16))


def _pack_weights(inputs):
    per_layer = []
    for li in range(L):
        Wq = np.asarray(inputs['Wq'][li][:, :, 0], np.float32) / 8.0
        Wk = np.asarray(inputs['Wk'][li][:, :, 0], np.float32)
        Wv = np.asarray(inputs['Wv'][li][:, :, 0], np.float32)
        Wo = np.asarray(inputs['Wo'][li][:, :, 0], np.float32)
        W1 = np.asarray(inputs['W1'][li], np.float32)  # [F, C, 3]
        W2 = np.asarray(inputs['W2'][li], np.float32)  # [C, F, 3]

        def packT(W):
            WT = W.T
            return np.concatenate([WT[kk * 128:(kk + 1) * 128, :] for kk in range(4)], axis=1)

        wq_p = packT(Wq); wk_p = packT(Wk); wo_p = packT(Wo); wv_p = packT(Wv)
        w1_p = np.zeros((128, 16 * 12 * 128), np.float32)
        for fm in range(16):
            for kk in range(4):
                for dk in range(3):
                    b = kk * 3 + dk
                    w1_p[:, fm * 1536 + b * 128: fm * 1536 + (b + 1) * 128] = \
                        W1[fm * 128:(fm + 1) * 128, kk * 128:(kk + 1) * 128, dk].T
        w2_p = np.zeros((128, 16 * 12 * 128), np.float32)
        for fk in range(16):
            for m in range(4):
                for dk in range(3):
                    b = fk * 12 + m * 3 + dk
                    w2_p[:, b * 128:(b + 1) * 128] = \
                        W2[m * 128:(m + 1) * 128, fk * 128:(fk + 1) * 128, dk].T
        par = np.zeros((128, 52), np.float32)

        def col4(vec):
            return np.asarray(vec, np.float32).reshape(4, 128).T

        par[:, 0:4] = col4(inputs['bq'][li]) / 8.0
        par[:, 4:8] = col4(inputs['bk'][li])
        par[:, 8:12] = col4(inputs['bo'][li])
        par[:, 12:28] = np.asarray(inputs['c1'][li], np.float32).reshape(16, 128).T
        par[:, 28:32] = col4(inputs['c2'][li])
        par[:, 32:36] = col4(inputs['g1'][li])
        par[:, 36:40] = col4(inputs['be1'][li])
        par[:, 40:44] = col4(inputs['g2'][li])
        par[:, 44:48] = col4(inputs['be2'][li])
        par[:, 48:52] = col4(inputs['bv'][li])
        per_layer.append(dict(wq=_f16(wq_p), wk=_f16(wk_p), wv=_f16(wv_p),
                              wo=_f16(wo_p), w1=_f16(w1_p), w2=_f16(w2_p), par=par))
    return per_layer


def kernel(**inputs):
    inputs = {k: np.asarray(v) for k, v in inputs.items()}
    x = inputs['x'].astype(np.float32) * inputs['x_mask'].astype(np.float32)
    has_bv = bool(np.any(inputs['bv'] != 0))
    ln1_aff = bool(np.any(inputs['g1'] != 1) or np.any(inputs['be1'] != 0))
    ln2_aff = bool(np.any(inputs['g2'] != 1) or np.any(inputs['be2'] != 0))
    flags = (has_bv, ln1_aff, ln2_aff)
    if flags not in _CACHE:
        _CACHE[flags] = build_program(flags)
    nc = _CACHE[flags]

    wl = _pack_weights(inputs)
    onesmat = np.concatenate([np.full((128, 128), -1.0 / 512, np.float32),
                              np.full((128, 128), 1.0 / 512, np.float32)], axis=1)

    in_maps = []
    for core in range(NC8):
        g, h = core // 2, core % 2
        t0 = h * TO
        cos_k, sin_k = _rope_tables(np.arange(t0, t0 + TO, dtype=np.float64))
        mh = np.ones((128, 514), np.float32)
        if h == 0:
            mh[:, 0:1] = 0
            hc = np.array([0.0, 0.0, 1.0, 0.0], np.float32)   # cl, -, cr, -
        else:
            mh[:, 513:514] = 0
            hc = np.array([1.0, 0.0, 0.0, 0.0], np.float32)
        im = {
            "x0": _f16(x[g][:, t0:t0 + TO]),
            "cos_k": _f16(cos_k), "sin_k": _f16(sin_k),
            "maskh": _f16(mh),
            "hcoef": np.repeat(hc[None, :], 128, axis=0),
            "onesmat": _f16(onesmat),
        }
        for li in range(L):
            w = wl[li]
            im[f"wq{li}"] = w['wq']; im[f"wk{li}"] = w['wk']
            im[f"wv{li}"] = w['wv']; im[f"wo{li}"] = w['wo']
            im[f"w1{li}"] = w['w1']; im[f"w2{li}"] = w['w2']
            im[f"par{li}"] = w['par']
        in_maps.append(im)

    global LAST_RESULT
    res = run_bass_kernel_spmd(nc, in_maps, core_ids=list(range(NC8)),
                               trace=TRACE)
    LAST_RESULT = res
    out = np.zeros((B, C, T), np.float32)
    for g in range(B):
        out[g, :, 0:TO] = res.results[2 * g]["out"]
        out[g, :, TO:T] = res.results[2 * g + 1]["out"]
    out_dt = np.asarray(inputs['x']).dtype
    return out.astype(out_dt)
